# revision 26
# baseline (speedup 1.0000x reference)
"""GCNConv (linear + edge-weighted gather + segment_sum) on 8 TRN2 NeuronCores.

Strategy (dst-sharded, per the 1D graph-partition hint):
- Phase 1: node rows sharded 8-way; each core computes h = x @ W.T + b for its
  6250 nodes (fp32 matmul, fp32 psum, bias add), casts to fp16 and AllGathers
  so every core holds the full h [50000, 256] in HBM.
- Phase 2: destination nodes are bin-packed into 50 bins/core (<=128 dsts per
  bin, edge counts balanced).  Edges land in 128-slot tiles (dst-segment runs,
  zero-weight padding).  dma_gather pulls h[src] rows (fp16, 512B each) into
  SBUF; a per-tile one-hot*weight matrix B1w [128,32] (host-built) reduces the
  128 gathered rows into <=32 segment partial sums on the TensorEngine; a
  second one-hot matrix B2 combines those chunk rows into the bin's 128 output
  rows.  int16 gather indices cap at 32767, so edges are split into a low
  stream (src < 32768) and a high stream (gather base offset 32768).
- Host work is layout only: sharding/transposes, edge sorting/binning, and
  scattering w_edge into the block-structured B1w/B2 operands.
"""

import sys

if "/opt/trn_rl_repo" not in sys.path:
    sys.path.insert(0, "/opt/trn_rl_repo")

import os

import numpy as np

N_NODES = 50000
N_EDGES = 800000
IN_DIM = 512
OUT_DIM = 256
NCORES = 8
NODES_PER_CORE = N_NODES // NCORES  # 6250
NB = 50                 # dst bins per core (each bin -> 128 output rows)
BINS = NCORES * NB      # 400
SPLIT = 32768           # int16 gather index limit
GCALL = int(os.environ.get("GCN_GCALL", "32"))  # tiles per dma_gather call
MAX_SEGS = 32           # segment columns per level-1 tile

f32 = np.float32
f16 = np.float16
i16 = np.int16


# ---------------------------------------------------------------- host prep

def _bin_pack(dst, is_h):
    """Assign each dst node to one of BINS bins (<=128 dsts each), balancing
    (low, high) edge counts.  Returns bin_of_node [N_NODES]."""
    l_cnt = np.bincount(dst[~is_h], minlength=N_NODES).astype(np.int64)
    h_cnt = np.bincount(dst[is_h], minlength=N_NODES).astype(np.int64)
    tot = l_cnt + h_cnt
    order = np.argsort(-tot, kind="stable")

    import heapq
    # heap of (load, nitems, bin_id); load balances total edges
    heap = [(0, 0, b) for b in range(BINS)]
    heapq.heapify(heap)
    bin_of = np.empty(N_NODES, dtype=np.int32)
    stash = []
    for node in order:
        while True:
            load, cnt, b = heapq.heappop(heap)
            if cnt < 128:
                break
            stash.append(None)  # full bin, drop it
        bin_of[node] = b
        heapq.heappush(heap, (load + int(tot[node]), cnt + 1, b))
    return bin_of, l_cnt, h_cnt


def _pack_tiles(seg_list):
    """Pack (m, idx_array, w_array) segments into 128-slot tiles.

    Returns list of tiles; each tile is (idx[128] int32, col[128] int8,
    w[128] f32, seg2m[32] int32 with -1 for unused).  Segments split freely at
    tile boundaries; a tile holds at most MAX_SEGS segments."""
    tiles = []
    cur_idx = np.zeros(128, np.int64)
    cur_col = np.zeros(128, np.int8)
    cur_w = np.zeros(128, f32)
    cur_s2m = np.full(MAX_SEGS, -1, np.int32)
    pos = 0
    nseg = 0

    def close():
        nonlocal pos, nseg, cur_idx, cur_col, cur_w, cur_s2m
        tiles.append((cur_idx, cur_col, cur_w, cur_s2m))
        cur_idx = np.zeros(128, np.int64)
        cur_col = np.zeros(128, np.int8)
        cur_w = np.zeros(128, f32)
        cur_s2m = np.full(MAX_SEGS, -1, np.int32)
        pos = 0
        nseg = 0

    for m, idxs, ws in seg_list:
        off = 0
        n = len(idxs)
        while off < n:
            if pos == 128 or nseg == MAX_SEGS:
                close()
            take = min(n - off, 128 - pos)
            cur_idx[pos:pos + take] = idxs[off:off + take]
            cur_col[pos:pos + take] = nseg
            cur_w[pos:pos + take] = ws[off:off + take]
            cur_s2m[nseg] = m
            nseg += 1
            pos += take
            off += take
    if pos > 0 or nseg > 0:
        close()
    return tiles


def _prep(x, W, b, w_edge, src, dst):
    """All host-side sharding/layout. Returns (params, in_maps, unshard)."""
    src = np.asarray(src).astype(np.int64)
    dst = np.asarray(dst).astype(np.int64)
    w_edge = np.asarray(w_edge).astype(f32)
    is_h = src >= SPLIT

    bin_of, _, _ = _bin_pack(dst, is_h)

    # order edges by (bin, dst, stream)
    ekey = np.lexsort((is_h, dst, bin_of[dst]))
    e_src = src[ekey]
    e_dst = dst[ekey]
    e_w = w_edge[ekey]
    e_h = is_h[ekey]
    e_bin = bin_of[e_dst]

    # per-bin segment lists -> tiles; assign every node (incl. degree-0) a
    # local row m within its bin up front
    tiles_L = [[] for _ in range(BINS)]   # per bin: list of tile tuples
    tiles_H = [[] for _ in range(BINS)]
    m_of_node = np.full(N_NODES, -1, np.int32)
    nodes_of_bin = [[] for _ in range(BINS)]
    for node in np.argsort(bin_of, kind="stable"):
        bn = bin_of[node]
        m_of_node[node] = len(nodes_of_bin[bn])
        nodes_of_bin[bn].append(int(node))

    # group edges by (bin, dst, stream) using run boundaries
    key = e_bin.astype(np.int64) * (N_NODES * 2) + e_dst * 2 + e_h
    bounds = np.flatnonzero(np.r_[True, key[1:] != key[:-1], True])

    seglists_L = [[] for _ in range(BINS)]
    seglists_H = [[] for _ in range(BINS)]
    for gi in range(len(bounds) - 1):
        s, e = bounds[gi], bounds[gi + 1]
        bn = int(e_bin[s])
        node = int(e_dst[s])
        m = int(m_of_node[node])
        if e_h[s]:
            seglists_H[bn].append((m, e_src[s:e] - SPLIT, e_w[s:e]))
        else:
            seglists_L[bn].append((m, e_src[s:e], e_w[s:e]))

    for bn in range(BINS):
        tiles_L[bn] = _pack_tiles(seglists_L[bn])
        tiles_H[bn] = _pack_tiles(seglists_H[bn])

    T1L = max(1, max(len(t) for t in tiles_L))
    T1H = max(1, max(len(t) for t in tiles_H))
    T1 = T1L + T1H
    T2 = -(-T1 // 4)              # level-2 rhs tiles per bin (ceil T1/4)
    n_chunks = 32 * T1

    pad_tile = (np.zeros(128, np.int64), np.zeros(128, np.int8),
                np.zeros(128, f32), np.full(MAX_SEGS, -1, np.int32))

    # build per-core arrays
    in_maps = []
    node_perm = np.empty(N_NODES, np.int64)   # final out row -> original dst
    valid = np.zeros(NCORES * NB * 128, bool)

    xT = np.ascontiguousarray(x.T)            # [512, 50000] f32
    WT = np.ascontiguousarray(W.T)            # [512, 256] f32
    b2d = np.ascontiguousarray(b[None, :])    # [1, 256] f32

    ncall_L = -(-(NB * T1L) // GCALL)
    ncall_H = -(-(NB * T1H) // GCALL)

    for c in range(NCORES):
        b1w = np.zeros((NB, 128, T1 * MAX_SEGS), f16)
        b2m = np.zeros((NB, 128, T2 * 128), f16)
        idx_L = np.zeros((NB * T1L, 128), np.int16)
        idx_H = np.zeros((NB * T1H, 128), np.int16)

        for lb in range(NB):
            bn = c * NB + lb
            tl = tiles_L[bn] + [pad_tile] * (T1L - len(tiles_L[bn]))
            th = tiles_H[bn] + [pad_tile] * (T1H - len(tiles_H[bn]))
            for t, (tidx, tcol, tw, ts2m) in enumerate(tl + th):
                if t < T1L:
                    idx_L[lb * T1L + t] = tidx.astype(np.int16)
                else:
                    idx_H[lb * T1H + (t - T1L)] = tidx.astype(np.int16)
                cols = t * MAX_SEGS + tcol.astype(np.int32)
                b1w[lb, np.arange(128), cols] = tw.astype(f16)
                # chunk ids t*32+s -> m
                for s2 in range(MAX_SEGS):
                    m = ts2m[s2]
                    if m >= 0:
                        ch = t * MAX_SEGS + s2
                        b2m[lb, ch % 128, (ch // 128) * 128 + m] = 1.0

            for m, node in enumerate(nodes_of_bin[bn]):
                row = c * NB * 128 + lb * 128 + m
                node_perm[node] = row
                valid[row] = True

        def wrap_calls(idx_tiles, ncall):
            flat = idx_tiles.reshape(-1)           # [ntiles*128]
            out = np.zeros((ncall, 128, GCALL * 8), np.int16)
            for k in range(ncall):
                chunk = flat[k * GCALL * 128:(k + 1) * GCALL * 128]
                buf = np.zeros(GCALL * 128, np.int16)
                buf[:len(chunk)] = chunk
                wrapped = buf.reshape(-1, 16).T    # [16, GCALL*8]
                out[k] = np.tile(wrapped, (8, 1))
            return out

        rows = slice(c * NODES_PER_CORE, (c + 1) * NODES_PER_CORE)
        in_maps.append({
            "xT": np.ascontiguousarray(xT[:, rows]),
            "WT": WT,
            "bias": b2d,
            "idx_l": wrap_calls(idx_L, ncall_L),
            "idx_h": wrap_calls(idx_H, ncall_H),
            "b1w": b1w,
            "b2m": b2m,
        })

    params = dict(T1L=T1L, T1H=T1H, T1=T1, T2=T2, n_chunks=n_chunks,
                  ncall_L=ncall_L, ncall_H=ncall_H,
                  ntile_L=NB * T1L, ntile_H=NB * T1H)

    def unshard(outs):
        full = np.concatenate([o.reshape(-1, OUT_DIM) for o in outs], axis=0)
        return np.ascontiguousarray(full[node_perm])

    return params, in_maps, unshard


# ---------------------------------------------------------------- device

def _build(p):
    import os
    import concourse.bass as bass
    import concourse.mybir as mybir
    import concourse.tile as tile
    from concourse import bacc

    STAGE = int(os.environ.get("GCN_STAGE", "4"))

    dt16 = mybir.dt.float16
    dt32 = mybir.dt.float32
    dti16 = mybir.dt.int16

    T1L, T1H, T1, T2 = p["T1L"], p["T1H"], p["T1"], p["T2"]
    ncall_L, ncall_H = p["ncall_L"], p["ncall_H"]
    ntile_L, ntile_H = p["ntile_L"], p["ntile_H"]

    nc = bacc.Bacc(None, target_bir_lowering=False, num_swdge_queues=2)
    trace_sim = os.environ.get("GCN_TRACESIM", "0") == "1"

    xT_in = nc.dram_tensor("xT", [IN_DIM, NODES_PER_CORE], dt32, kind="ExternalInput")
    WT_in = nc.dram_tensor("WT", [IN_DIM, OUT_DIM], dt32, kind="ExternalInput")
    b_in = nc.dram_tensor("bias", [1, OUT_DIM], dt32, kind="ExternalInput")
    idxl_in = nc.dram_tensor("idx_l", [ncall_L, 128, GCALL * 8], dti16, kind="ExternalInput")
    idxh_in = nc.dram_tensor("idx_h", [ncall_H, 128, GCALL * 8], dti16, kind="ExternalInput")
    b1w_in = nc.dram_tensor("b1w", [NB, 128, T1 * MAX_SEGS], dt16, kind="ExternalInput")
    b2m_in = nc.dram_tensor("b2m", [NB, 128, T2 * 128], dt16, kind="ExternalInput")
    out_dr = nc.dram_tensor("out", [NB * 128, OUT_DIM], dt32, kind="ExternalOutput")

    h_loc = nc.dram_tensor("h_loc", [NODES_PER_CORE, OUT_DIM], dt16)
    h_all = nc.dram_tensor("h_all", [N_NODES, OUT_DIM], dt16, addr_space="Shared")

    NT = 49  # node tiles per core: 48*128 + 106 = 6250

    with tile.TileContext(nc, trace_sim=trace_sim) as tc:
        # ---------------- phase 1: h = x @ W.T + b (fp32), cast f16
        with (
            tc.tile_pool(name="p1", bufs=1) as p1,
            tc.tile_pool(name="p1x", bufs=1) as p1x,
            tc.tile_pool(name="p1h", bufs=4) as p1h,
            tc.tile_pool(name="ps1", bufs=2, space="PSUM") as ps1,
        ):
            wt_sb = []
            for k in range(4):
                t = p1.tile([128, OUT_DIM], dt16, tag=f"wt{k}")
                nc.gpsimd.dma_start(t[:], WT_in[128 * k:128 * (k + 1), :])
                wt_sb.append(t)
            bias_sb = p1.tile([128, OUT_DIM], dt32, tag="bias")
            nc.sync.dma_start(
                bias_sb[:],
                bass.AP(tensor=b_in.ap().tensor, offset=0,
                        ap=[[0, 128]] + [list(b_in.ap().ap[-1])]),
            )
            xt_big = []
            for k in range(4):
                xt = p1x.tile([128, NODES_PER_CORE], dt16, tag=f"xt{k}")
                nc.gpsimd.dma_start(xt[:], xT_in[128 * k:128 * (k + 1), :])
                xt_big.append(xt)
            for ntt in range(NT):
                w = min(128, NODES_PER_CORE - ntt * 128)
                hp = ps1.tile([128, OUT_DIM], dt32, tag="hps")
                for k in range(4):
                    nc.tensor.matmul(
                        hp[:w, :],
                        xt_big[k][:, ntt * 128:ntt * 128 + w],
                        wt_sb[k][:],
                        start=(k == 0), stop=(k == 3))
                hv = p1h.tile([128, OUT_DIM], dt16, tag="hv")
                nc.vector.tensor_add(hv[:w, :], hp[:w, :], bias_sb[:w, :])
                nc.sync.dma_start(h_loc[ntt * 128:ntt * 128 + w, :], hv[:w, :])

        # ---------------- AllGather
        if STAGE >= 1:
            nc.gpsimd.collective_compute(
                "AllGather",
                mybir.AluOpType.bypass,
                replica_groups=[list(range(NCORES))],
                ins=[h_loc.ap().opt()],
                outs=[h_all.ap().opt()],
            )

        # ---------------- phase 2: gather + two-level segment matmuls
        with (
            tc.tile_pool(name="gl", bufs=int(os.environ.get("GCN_GBUFS", "3"))) as gl_pool,
            tc.tile_pool(name="gh", bufs=int(os.environ.get("GCN_GBUFS", "3"))) as gh_pool,
            tc.tile_pool(name="ixp", bufs=3) as ix_pool,
            tc.tile_pool(name="bp", bufs=3) as b_pool,
            tc.tile_pool(name="ck", bufs=2 * (T2 + 1)) as ck_pool,
            tc.tile_pool(name="op", bufs=3) as out_pool,
            tc.tile_pool(name="ps2", bufs=3, space="PSUM") as ps2,
            tc.tile_pool(name="ps3", bufs=2, space="PSUM") as ps3,
        ):
            gtiles = {"L": [], "H": []}

            def issue_gather(stream, k):
                ncall, ntile, idx_dr, base = {
                    "L": (ncall_L, ntile_L, idxl_in, 0),
                    "H": (ncall_H, ntile_H, idxh_in, SPLIT),
                }[stream]
                nt = min(GCALL, ntile - k * GCALL)
                it = ix_pool.tile([128, GCALL * 8], dti16, tag="ix")
                nc.scalar.dma_start(it[:], idx_dr[k, :, :])
                pool = gl_pool if stream == "L" else gh_pool
                gt = pool.tile([128, GCALL, OUT_DIM], dt16, tag="g" + stream)
                nc.gpsimd.dma_gather(
                    gt[:, :nt, :],
                    h_all[base:, :],
                    it[:, :nt * 8],
                    num_idxs=nt * 128,
                    num_idxs_reg=nt * 128,
                    elem_size=OUT_DIM,
                    single_packet=os.environ.get("GCN_SP", "0") == "1",
                    queue_num=0 if stream == "L" else 1,
                )
                gtiles[stream].append(gt)

            def get_tile_ap(stream, g):
                # slot data for global tile g of a stream
                return gtiles[stream][g // GCALL][:, g % GCALL, :]

            NB_RUN = NB if STAGE >= 2 else 0
            for lb in range(NB_RUN):
                # pull gather calls needed for this bin
                while len(gtiles["L"]) * GCALL < min((lb + 1) * T1L, ntile_L) \
                        or len(gtiles["L"]) == 0:
                    issue_gather("L", len(gtiles["L"]))
                while len(gtiles["H"]) * GCALL < min((lb + 1) * T1H, ntile_H) \
                        or len(gtiles["H"]) == 0:
                    issue_gather("H", len(gtiles["H"]))

                if STAGE == 2:
                    # just land the gathered tiles in out rows (garbage data,
                    # exercises gathers + DMA out)
                    ot = out_pool.tile([128, OUT_DIM], dt32, tag="ot")
                    g0 = gtiles["L"][(lb * T1L) // GCALL][:, (lb * T1L) % GCALL, :]
                    nc.vector.tensor_copy(ot[:], g0)
                    nc.sync.dma_start(out_dr[lb * 128:(lb + 1) * 128, :], ot[:])
                    continue

                b1t = b_pool.tile([128, T1 * MAX_SEGS], dt16, tag="b1")
                nc.scalar.dma_start(b1t[:], b1w_in[lb, :, :])
                b2t = b_pool.tile([128, T2 * 128], dt16, tag="b2")
                nc.scalar.dma_start(b2t[:], b2m_in[lb, :, :])

                # level 1
                cktiles = []
                for grp in range(T2):
                    g_n = min(4, T1 - grp * 4)
                    cps = ps2.tile([128, OUT_DIM], dt32, tag="cps")
                    for r in range(g_n):
                        t = grp * 4 + r
                        if t < T1L:
                            rhs = get_tile_ap("L", lb * T1L + t)
                        else:
                            rhs = get_tile_ap("H", lb * T1H + (t - T1L))
                        nc.tensor.matmul(
                            cps[32 * r:32 * (r + 1), :],
                            b1t[:, t * MAX_SEGS:(t + 1) * MAX_SEGS],
                            rhs,
                            start=True, stop=True,
                            tile_position=(0, 32 * r),
                        )
                    ckt = ck_pool.tile([128, OUT_DIM], dt16, tag="ck")
                    nc.any.tensor_copy(ckt[:32 * g_n, :], cps[:32 * g_n, :])
                    cktiles.append((ckt, 32 * g_n))

                if STAGE == 3:
                    ot = out_pool.tile([128, OUT_DIM], dt32, tag="ot")
                    nc.vector.tensor_copy(ot[:], cktiles[0][0][:])
                    nc.sync.dma_start(out_dr[lb * 128:(lb + 1) * 128, :], ot[:])
                    continue

                # level 2
                ops = ps3.tile([128, OUT_DIM], dt32, tag="ops")
                for j, (ckt, kk) in enumerate(cktiles):
                    nc.tensor.matmul(
                        ops[:],
                        b2t[:kk, j * 128:(j + 1) * 128],
                        ckt[:kk, :],
                        start=(j == 0), stop=(j == len(cktiles) - 1),
                    )
                ot = out_pool.tile([128, OUT_DIM], dt32, tag="ot")
                nc.any.tensor_copy(ot[:], ops[:])
                nc.sync.dma_start(out_dr[lb * 128:(lb + 1) * 128, :], ot[:])

    nc.compile()
    return nc


# ---------------------------------------------------------------- entry

TRACE = False          # test harness can flip this for neuron-profile timing
LAST_RESULT = None
_LAST_BUILD = None


def kernel(x, W, b, w_edge, src, dst):
    global LAST_RESULT, _LAST_BUILD
    from concourse.bass_utils import run_bass_kernel_spmd

    x = np.asarray(x, dtype=f32)
    W = np.asarray(W, dtype=f32)
    b = np.asarray(b, dtype=f32)

    params, in_maps, unshard = _prep(x, W, b, w_edge, src, dst)
    nc = _build(params)
    _LAST_BUILD = (nc, in_maps)
    res = run_bass_kernel_spmd(nc, in_maps, core_ids=list(range(NCORES)),
                               trace=TRACE)
    LAST_RESULT = res
    outs = [res.results[c]["out"] for c in range(NCORES)]
    return unshard(outs)


def bench(iters=32):
    """Time device-resident executions of the compiled kernel (no host I/O).

    Returns (batched_ns, min_iter_ns): batched = enqueue `iters` executions
    then sync once (pipelined, amortizes RPC); min_iter = best single
    dispatch+exec+sync round trip."""
    import time
    import jax
    from jax.sharding import Mesh, PartitionSpec
    from jax.experimental.shard_map import shard_map
    from concourse import bass2jax, mybir

    nc, in_maps = _LAST_BUILD
    bass2jax.install_neuronx_cc_hook()

    part_name = nc.partition_id_tensor.name if nc.partition_id_tensor else None
    in_names, out_names, out_avals, zeros = [], [], [], []
    for alloc in nc.m.functions[0].allocations:
        if not isinstance(alloc, mybir.MemoryLocationSet):
            continue
        name = alloc.memorylocations[0].name
        if alloc.kind == "ExternalInput":
            if name != part_name:
                in_names.append(name)
        elif alloc.kind == "ExternalOutput":
            out_names.append(name)
            shape = tuple(alloc.tensor_shape)
            dtype = mybir.dt.np(alloc.dtype)
            out_avals.append(jax.core.ShapedArray(shape, dtype))
            zeros.append(np.zeros(shape, dtype))
    n_params = len(in_names)
    all_names = in_names + out_names
    if part_name is not None:
        all_names = all_names + [part_name]

    def _body(*args):
        operands = list(args)
        if part_name is not None:
            operands.append(bass2jax.partition_id_tensor())
        outs = bass2jax._bass_exec_p.bind(
            *operands,
            out_avals=tuple(out_avals),
            in_names=tuple(all_names),
            out_names=tuple(out_names),
            lowering_input_output_aliases=(),
            sim_require_finite=True,
            sim_require_nnan=True,
            nc=nc,
        )
        return tuple(outs)

    devices = jax.devices()[:NCORES]
    mesh = Mesh(np.asarray(devices), ("core",))
    nin = n_params + len(out_names)
    fn = jax.jit(shard_map(
        _body, mesh=mesh,
        in_specs=(PartitionSpec("core"),) * nin,
        out_specs=(PartitionSpec("core"),) * len(out_names),
        check_rep=False), keep_unused=True)

    sharding = jax.sharding.NamedSharding(mesh, PartitionSpec("core"))
    args = []
    for i, name in enumerate(in_names):
        cat = np.concatenate([np.asarray(m[name]) for m in in_maps], axis=0)
        args.append(jax.device_put(cat, sharding))
    for z in zeros:
        cat = np.zeros((NCORES * z.shape[0], *z.shape[1:]), z.dtype)
        args.append(jax.device_put(cat, sharding))

    out = fn(*args)          # warmup / compile
    jax.block_until_ready(out)
    out = fn(*args)
    jax.block_until_ready(out)

    t0 = time.perf_counter()
    outs = [fn(*args) for _ in range(iters)]
    jax.block_until_ready(outs)
    batched = (time.perf_counter() - t0) / iters

    best = float("inf")
    for _ in range(8):
        t0 = time.perf_counter()
        jax.block_until_ready(fn(*args))
        best = min(best, time.perf_counter() - t0)

    return int(batched * 1e9), int(best * 1e9)


if __name__ == "__main__":
    rng = np.random.default_rng(0)
    x = rng.standard_normal((N_NODES, IN_DIM), dtype=f32)
    W = (rng.standard_normal((OUT_DIM, IN_DIM), dtype=f32) / np.sqrt(IN_DIM)).astype(f32)
    b = (rng.standard_normal(OUT_DIM, dtype=f32) * 0.01).astype(f32)
    w_edge = rng.random(N_EDGES, dtype=f32)
    src = rng.integers(0, N_NODES, N_EDGES, dtype=np.int64)
    dst = rng.integers(0, N_NODES, N_EDGES, dtype=np.int64)
    out = kernel(x=x, W=W, b=b, w_edge=w_edge, src=src, dst=dst)
    h = x @ W.T + b
    import scipy.sparse as sp  # noqa — may not exist; fallback below
    try:
        A = sp.coo_matrix((w_edge, (dst, src)), shape=(N_NODES, N_NODES)).tocsr()
        want = A @ h
    except Exception:
        want = np.zeros_like(h)
        np.add.at(want, dst, h[src] * w_edge[:, None])
    err = np.abs(out - want).max() / (np.abs(want).max() + 1e-9)
    print("rel err:", err)


# revision 27
# speedup vs baseline: 1.1426x; 1.1426x over previous
"""GCNConv (linear + edge-weighted gather + segment_sum) on 8 TRN2 NeuronCores.

Strategy (dst-sharded, per the 1D graph-partition hint):
- Phase 1: node rows sharded 8-way; each core computes h = x @ W.T + b for its
  6250 nodes (fp32 matmul, fp32 psum, bias add), casts to fp16 and AllGathers
  so every core holds the full h [50000, 256] in HBM.
- Phase 2: destination nodes are bin-packed into 50 bins/core (<=128 dsts per
  bin, edge counts balanced).  Edges land in 128-slot tiles (dst-segment runs,
  zero-weight padding).  dma_gather pulls h[src] rows (fp16, 512B each) into
  SBUF; a per-tile one-hot*weight matrix B1w [128,32] (host-built) reduces the
  128 gathered rows into <=32 segment partial sums on the TensorEngine; a
  second one-hot matrix B2 combines those chunk rows into the bin's 128 output
  rows.  int16 gather indices cap at 32767, so edges are split into a low
  stream (src < 32768) and a high stream (gather base offset 32768).
- Host work is layout only: sharding/transposes, edge sorting/binning, and
  scattering w_edge into the block-structured B1w/B2 operands.
"""

import sys

if "/opt/trn_rl_repo" not in sys.path:
    sys.path.insert(0, "/opt/trn_rl_repo")

import os

import numpy as np

N_NODES = 50000
N_EDGES = 800000
IN_DIM = 512
OUT_DIM = 256
NCORES = 8
NODES_PER_CORE = N_NODES // NCORES  # 6250
NB = 50                 # dst bins per core (each bin -> 128 output rows)
BINS = NCORES * NB      # 400
SPLIT = 32768           # int16 gather index limit
GCALL = int(os.environ.get("GCN_GCALL", "32"))  # tiles per dma_gather call
MAX_SEGS = 32           # segment columns per level-1 tile

f32 = np.float32
f16 = np.float16
i16 = np.int16


# ---------------------------------------------------------------- host prep

def _bin_pack(dst, is_h):
    """Assign each dst node to one of BINS bins (<=128 dsts each), balancing
    (low, high) edge counts.  Returns bin_of_node [N_NODES]."""
    l_cnt = np.bincount(dst[~is_h], minlength=N_NODES).astype(np.int64)
    h_cnt = np.bincount(dst[is_h], minlength=N_NODES).astype(np.int64)
    tot = l_cnt + h_cnt
    order = np.argsort(-tot, kind="stable")

    import heapq
    # heap of (load, nitems, bin_id); load balances total edges
    heap = [(0, 0, b) for b in range(BINS)]
    heapq.heapify(heap)
    bin_of = np.empty(N_NODES, dtype=np.int32)
    stash = []
    for node in order:
        while True:
            load, cnt, b = heapq.heappop(heap)
            if cnt < 128:
                break
            stash.append(None)  # full bin, drop it
        bin_of[node] = b
        heapq.heappush(heap, (load + int(tot[node]), cnt + 1, b))
    return bin_of, l_cnt, h_cnt


def _pack_tiles(seg_list):
    """Pack (m, idx_array, w_array) segments into 128-slot tiles.

    Returns list of tiles; each tile is (idx[128] int32, col[128] int8,
    w[128] f32, seg2m[32] int32 with -1 for unused).  Segments split freely at
    tile boundaries; a tile holds at most MAX_SEGS segments."""
    tiles = []
    cur_idx = np.zeros(128, np.int64)
    cur_col = np.zeros(128, np.int8)
    cur_w = np.zeros(128, f32)
    cur_s2m = np.full(MAX_SEGS, -1, np.int32)
    pos = 0
    nseg = 0

    def close():
        nonlocal pos, nseg, cur_idx, cur_col, cur_w, cur_s2m
        tiles.append((cur_idx, cur_col, cur_w, cur_s2m))
        cur_idx = np.zeros(128, np.int64)
        cur_col = np.zeros(128, np.int8)
        cur_w = np.zeros(128, f32)
        cur_s2m = np.full(MAX_SEGS, -1, np.int32)
        pos = 0
        nseg = 0

    for m, idxs, ws in seg_list:
        off = 0
        n = len(idxs)
        while off < n:
            if pos == 128 or nseg == MAX_SEGS:
                close()
            take = min(n - off, 128 - pos)
            cur_idx[pos:pos + take] = idxs[off:off + take]
            cur_col[pos:pos + take] = nseg
            cur_w[pos:pos + take] = ws[off:off + take]
            cur_s2m[nseg] = m
            nseg += 1
            pos += take
            off += take
    if pos > 0 or nseg > 0:
        close()
    return tiles


def _prep(x, W, b, w_edge, src, dst):
    """All host-side sharding/layout. Returns (params, in_maps, unshard)."""
    src = np.asarray(src).astype(np.int64)
    dst = np.asarray(dst).astype(np.int64)
    w_edge = np.asarray(w_edge).astype(f32)
    is_h = src >= SPLIT

    bin_of, _, _ = _bin_pack(dst, is_h)

    # order edges by (bin, dst, stream)
    ekey = np.lexsort((is_h, dst, bin_of[dst]))
    e_src = src[ekey]
    e_dst = dst[ekey]
    e_w = w_edge[ekey]
    e_h = is_h[ekey]
    e_bin = bin_of[e_dst]

    # per-bin segment lists -> tiles; assign every node (incl. degree-0) a
    # local row m within its bin up front
    tiles_L = [[] for _ in range(BINS)]   # per bin: list of tile tuples
    tiles_H = [[] for _ in range(BINS)]
    m_of_node = np.full(N_NODES, -1, np.int32)
    nodes_of_bin = [[] for _ in range(BINS)]
    for node in np.argsort(bin_of, kind="stable"):
        bn = bin_of[node]
        m_of_node[node] = len(nodes_of_bin[bn])
        nodes_of_bin[bn].append(int(node))

    # group edges by (bin, dst, stream) using run boundaries
    key = e_bin.astype(np.int64) * (N_NODES * 2) + e_dst * 2 + e_h
    bounds = np.flatnonzero(np.r_[True, key[1:] != key[:-1], True])

    seglists_L = [[] for _ in range(BINS)]
    seglists_H = [[] for _ in range(BINS)]
    for gi in range(len(bounds) - 1):
        s, e = bounds[gi], bounds[gi + 1]
        bn = int(e_bin[s])
        node = int(e_dst[s])
        m = int(m_of_node[node])
        if e_h[s]:
            seglists_H[bn].append((m, e_src[s:e] - SPLIT, e_w[s:e]))
        else:
            seglists_L[bn].append((m, e_src[s:e], e_w[s:e]))

    for bn in range(BINS):
        tiles_L[bn] = _pack_tiles(seglists_L[bn])
        tiles_H[bn] = _pack_tiles(seglists_H[bn])

    T1L = max(1, max(len(t) for t in tiles_L))
    T1H = max(1, max(len(t) for t in tiles_H))
    T1 = T1L + T1H
    T2 = -(-T1 // 4)              # level-2 rhs tiles per bin (ceil T1/4)
    n_chunks = 32 * T1

    pad_tile = (np.zeros(128, np.int64), np.zeros(128, np.int8),
                np.zeros(128, f32), np.full(MAX_SEGS, -1, np.int32))

    # build per-core arrays
    in_maps = []
    node_perm = np.empty(N_NODES, np.int64)   # final out row -> original dst
    valid = np.zeros(NCORES * NB * 128, bool)

    xT = np.ascontiguousarray(x.T)            # [512, 50000] f32
    WT = np.ascontiguousarray(W.T)            # [512, 256] f32
    b2d = np.ascontiguousarray(b[None, :])    # [1, 256] f32

    ncall_L = -(-(NB * T1L) // GCALL)
    ncall_H = -(-(NB * T1H) // GCALL)

    for c in range(NCORES):
        b1w = np.zeros((NB, 128, T1 * MAX_SEGS), f16)
        b2m = np.zeros((NB, 128, T2 * 128), f16)
        idx_L = np.zeros((NB * T1L, 128), np.int16)
        idx_H = np.zeros((NB * T1H, 128), np.int16)

        for lb in range(NB):
            bn = c * NB + lb
            tl = tiles_L[bn] + [pad_tile] * (T1L - len(tiles_L[bn]))
            th = tiles_H[bn] + [pad_tile] * (T1H - len(tiles_H[bn]))
            for t, (tidx, tcol, tw, ts2m) in enumerate(tl + th):
                if t < T1L:
                    idx_L[lb * T1L + t] = tidx.astype(np.int16)
                else:
                    idx_H[lb * T1H + (t - T1L)] = tidx.astype(np.int16)
                cols = t * MAX_SEGS + tcol.astype(np.int32)
                b1w[lb, np.arange(128), cols] = tw.astype(f16)
                # chunk ids t*32+s -> m
                for s2 in range(MAX_SEGS):
                    m = ts2m[s2]
                    if m >= 0:
                        ch = t * MAX_SEGS + s2
                        b2m[lb, ch % 128, (ch // 128) * 128 + m] = 1.0

            for m, node in enumerate(nodes_of_bin[bn]):
                row = c * NB * 128 + lb * 128 + m
                node_perm[node] = row
                valid[row] = True

        def wrap_calls(idx_tiles, ncall):
            flat = idx_tiles.reshape(-1)           # [ntiles*128]
            out = np.zeros((ncall, 128, GCALL * 8), np.int16)
            for k in range(ncall):
                chunk = flat[k * GCALL * 128:(k + 1) * GCALL * 128]
                buf = np.zeros(GCALL * 128, np.int16)
                buf[:len(chunk)] = chunk
                wrapped = buf.reshape(-1, 16).T    # [16, GCALL*8]
                out[k] = np.tile(wrapped, (8, 1))
            return out

        rows = slice(c * NODES_PER_CORE, (c + 1) * NODES_PER_CORE)
        in_maps.append({
            "xT": np.ascontiguousarray(xT[:, rows]),
            "WT": WT,
            "bias": b2d,
            "idx_l": wrap_calls(idx_L, ncall_L),
            "idx_h": wrap_calls(idx_H, ncall_H),
            "b1w": b1w,
            "b2m": b2m,
        })

    params = dict(T1L=T1L, T1H=T1H, T1=T1, T2=T2, n_chunks=n_chunks,
                  ncall_L=ncall_L, ncall_H=ncall_H,
                  ntile_L=NB * T1L, ntile_H=NB * T1H)

    def unshard(outs):
        full = np.concatenate([o.reshape(-1, OUT_DIM) for o in outs], axis=0)
        return np.ascontiguousarray(full[node_perm])

    return params, in_maps, unshard


# ---------------------------------------------------------------- device

def _build(p):
    import os
    import concourse.bass as bass
    import concourse.mybir as mybir
    import concourse.tile as tile
    from concourse import bacc

    STAGE = int(os.environ.get("GCN_STAGE", "4"))

    dt16 = mybir.dt.float16
    dt32 = mybir.dt.float32
    dti16 = mybir.dt.int16

    T1L, T1H, T1, T2 = p["T1L"], p["T1H"], p["T1"], p["T2"]
    ncall_L, ncall_H = p["ncall_L"], p["ncall_H"]
    ntile_L, ntile_H = p["ntile_L"], p["ntile_H"]

    nc = bacc.Bacc(None, target_bir_lowering=False, num_swdge_queues=4)
    trace_sim = os.environ.get("GCN_TRACESIM", "0") == "1"

    xT_in = nc.dram_tensor("xT", [IN_DIM, NODES_PER_CORE], dt32, kind="ExternalInput")
    WT_in = nc.dram_tensor("WT", [IN_DIM, OUT_DIM], dt32, kind="ExternalInput")
    b_in = nc.dram_tensor("bias", [1, OUT_DIM], dt32, kind="ExternalInput")
    idxl_in = nc.dram_tensor("idx_l", [ncall_L, 128, GCALL * 8], dti16, kind="ExternalInput")
    idxh_in = nc.dram_tensor("idx_h", [ncall_H, 128, GCALL * 8], dti16, kind="ExternalInput")
    b1w_in = nc.dram_tensor("b1w", [NB, 128, T1 * MAX_SEGS], dt16, kind="ExternalInput")
    b2m_in = nc.dram_tensor("b2m", [NB, 128, T2 * 128], dt16, kind="ExternalInput")
    out_dr = nc.dram_tensor("out", [NB * 128, OUT_DIM], dt32, kind="ExternalOutput")

    h_loc = nc.dram_tensor("h_loc", [NODES_PER_CORE, OUT_DIM], dt16)
    h_all = nc.dram_tensor("h_all", [N_NODES, OUT_DIM], dt16, addr_space="Shared")

    NT = 49  # node tiles per core: 48*128 + 106 = 6250

    with tile.TileContext(nc, trace_sim=trace_sim) as tc:
        # ---------------- phase 1: h = x @ W.T + b (fp32), cast f16
        with (
            tc.tile_pool(name="p1", bufs=1) as p1,
            tc.tile_pool(name="p1x", bufs=1) as p1x,
            tc.tile_pool(name="p1h", bufs=4) as p1h,
            tc.tile_pool(name="ps1", bufs=2, space="PSUM") as ps1,
        ):
            wt_sb = []
            for k in range(4):
                t = p1.tile([128, OUT_DIM], dt16, tag=f"wt{k}")
                nc.gpsimd.dma_start(t[:], WT_in[128 * k:128 * (k + 1), :])
                wt_sb.append(t)
            bias_sb = p1.tile([128, OUT_DIM], dt32, tag="bias")
            nc.sync.dma_start(
                bias_sb[:],
                bass.AP(tensor=b_in.ap().tensor, offset=0,
                        ap=[[0, 128]] + [list(b_in.ap().ap[-1])]),
            )
            xt_big = []
            for k in range(4):
                xt = p1x.tile([128, NODES_PER_CORE], dt16, tag=f"xt{k}")
                nc.gpsimd.dma_start(xt[:], xT_in[128 * k:128 * (k + 1), :])
                xt_big.append(xt)
            for ntt in range(NT):
                w = min(128, NODES_PER_CORE - ntt * 128)
                hp = ps1.tile([128, OUT_DIM], dt32, tag="hps")
                for k in range(4):
                    nc.tensor.matmul(
                        hp[:w, :],
                        xt_big[k][:, ntt * 128:ntt * 128 + w],
                        wt_sb[k][:],
                        start=(k == 0), stop=(k == 3))
                hv = p1h.tile([128, OUT_DIM], dt16, tag="hv")
                nc.vector.tensor_add(hv[:w, :], hp[:w, :], bias_sb[:w, :])
                nc.sync.dma_start(h_loc[ntt * 128:ntt * 128 + w, :], hv[:w, :])

        # ---------------- AllGather
        if STAGE >= 1:
            nc.gpsimd.collective_compute(
                "AllGather",
                mybir.AluOpType.bypass,
                replica_groups=[list(range(NCORES))],
                ins=[h_loc.ap().opt()],
                outs=[h_all.ap().opt()],
            )

        # ---------------- phase 2: gather + two-level segment matmuls
        with (
            tc.tile_pool(name="gl", bufs=int(os.environ.get("GCN_GBUFS", "3"))) as gl_pool,
            tc.tile_pool(name="gh", bufs=int(os.environ.get("GCN_GBUFS", "3"))) as gh_pool,
            tc.tile_pool(name="ixp", bufs=3) as ix_pool,
            tc.tile_pool(name="bp", bufs=3) as b_pool,
            tc.tile_pool(name="ck", bufs=2 * (T2 + 1)) as ck_pool,
            tc.tile_pool(name="op", bufs=3) as out_pool,
            tc.tile_pool(name="ps2", bufs=3, space="PSUM") as ps2,
            tc.tile_pool(name="ps3", bufs=2, space="PSUM") as ps3,
        ):
            gtiles = {"L": [], "H": []}
            qrr = [0]

            def issue_gather(stream, k):
                ncall, ntile, idx_dr, base = {
                    "L": (ncall_L, ntile_L, idxl_in, 0),
                    "H": (ncall_H, ntile_H, idxh_in, SPLIT),
                }[stream]
                nt = min(GCALL, ntile - k * GCALL)
                it = ix_pool.tile([128, GCALL * 8], dti16, tag="ix")
                nc.scalar.dma_start(it[:], idx_dr[k, :, :])
                pool = gl_pool if stream == "L" else gh_pool
                gt = pool.tile([128, GCALL, OUT_DIM], dt16, tag="g" + stream)
                nc.gpsimd.dma_gather(
                    gt[:, :nt, :],
                    h_all[base:, :],
                    it[:, :nt * 8],
                    num_idxs=nt * 128,
                    num_idxs_reg=nt * 128,
                    elem_size=OUT_DIM,
                    single_packet=os.environ.get("GCN_SP", "0") == "1",
                    queue_num=qrr[0] % 4,
                )
                qrr[0] += 1
                gtiles[stream].append(gt)

            def get_tile_ap(stream, g):
                # slot data for global tile g of a stream
                return gtiles[stream][g // GCALL][:, g % GCALL, :]

            NB_RUN = NB if STAGE >= 2 else 0
            for lb in range(NB_RUN):
                # pull gather calls needed for this bin
                while len(gtiles["L"]) * GCALL < min((lb + 1) * T1L, ntile_L) \
                        or len(gtiles["L"]) == 0:
                    issue_gather("L", len(gtiles["L"]))
                while len(gtiles["H"]) * GCALL < min((lb + 1) * T1H, ntile_H) \
                        or len(gtiles["H"]) == 0:
                    issue_gather("H", len(gtiles["H"]))

                if STAGE == 2:
                    # just land the gathered tiles in out rows (garbage data,
                    # exercises gathers + DMA out)
                    ot = out_pool.tile([128, OUT_DIM], dt32, tag="ot")
                    g0 = gtiles["L"][(lb * T1L) // GCALL][:, (lb * T1L) % GCALL, :]
                    nc.vector.tensor_copy(ot[:], g0)
                    nc.sync.dma_start(out_dr[lb * 128:(lb + 1) * 128, :], ot[:])
                    continue

                b1t = b_pool.tile([128, T1 * MAX_SEGS], dt16, tag="b1")
                nc.scalar.dma_start(b1t[:], b1w_in[lb, :, :])
                b2t = b_pool.tile([128, T2 * 128], dt16, tag="b2")
                nc.scalar.dma_start(b2t[:], b2m_in[lb, :, :])

                # level 1
                cktiles = []
                for grp in range(T2):
                    g_n = min(4, T1 - grp * 4)
                    cps = ps2.tile([128, OUT_DIM], dt32, tag="cps")
                    for r in range(g_n):
                        t = grp * 4 + r
                        if t < T1L:
                            rhs = get_tile_ap("L", lb * T1L + t)
                        else:
                            rhs = get_tile_ap("H", lb * T1H + (t - T1L))
                        nc.tensor.matmul(
                            cps[32 * r:32 * (r + 1), :],
                            b1t[:, t * MAX_SEGS:(t + 1) * MAX_SEGS],
                            rhs,
                            start=True, stop=True,
                            tile_position=(0, 32 * r),
                        )
                    ckt = ck_pool.tile([128, OUT_DIM], dt16, tag="ck")
                    nc.any.tensor_copy(ckt[:32 * g_n, :], cps[:32 * g_n, :])
                    cktiles.append((ckt, 32 * g_n))

                if STAGE == 3:
                    ot = out_pool.tile([128, OUT_DIM], dt32, tag="ot")
                    nc.vector.tensor_copy(ot[:], cktiles[0][0][:])
                    nc.sync.dma_start(out_dr[lb * 128:(lb + 1) * 128, :], ot[:])
                    continue

                # level 2
                ops = ps3.tile([128, OUT_DIM], dt32, tag="ops")
                for j, (ckt, kk) in enumerate(cktiles):
                    nc.tensor.matmul(
                        ops[:],
                        b2t[:kk, j * 128:(j + 1) * 128],
                        ckt[:kk, :],
                        start=(j == 0), stop=(j == len(cktiles) - 1),
                    )
                ot = out_pool.tile([128, OUT_DIM], dt32, tag="ot")
                nc.any.tensor_copy(ot[:], ops[:])
                nc.sync.dma_start(out_dr[lb * 128:(lb + 1) * 128, :], ot[:])

    nc.compile()
    return nc


# ---------------------------------------------------------------- entry

TRACE = False          # test harness can flip this for neuron-profile timing
LAST_RESULT = None
_LAST_BUILD = None


def kernel(x, W, b, w_edge, src, dst):
    global LAST_RESULT, _LAST_BUILD
    from concourse.bass_utils import run_bass_kernel_spmd

    x = np.asarray(x, dtype=f32)
    W = np.asarray(W, dtype=f32)
    b = np.asarray(b, dtype=f32)

    params, in_maps, unshard = _prep(x, W, b, w_edge, src, dst)
    nc = _build(params)
    _LAST_BUILD = (nc, in_maps)
    res = run_bass_kernel_spmd(nc, in_maps, core_ids=list(range(NCORES)),
                               trace=TRACE)
    LAST_RESULT = res
    outs = [res.results[c]["out"] for c in range(NCORES)]
    return unshard(outs)


def bench(iters=32):
    """Time device-resident executions of the compiled kernel (no host I/O).

    Returns (batched_ns, min_iter_ns): batched = enqueue `iters` executions
    then sync once (pipelined, amortizes RPC); min_iter = best single
    dispatch+exec+sync round trip."""
    import time
    import jax
    from jax.sharding import Mesh, PartitionSpec
    from jax.experimental.shard_map import shard_map
    from concourse import bass2jax, mybir

    nc, in_maps = _LAST_BUILD
    bass2jax.install_neuronx_cc_hook()

    part_name = nc.partition_id_tensor.name if nc.partition_id_tensor else None
    in_names, out_names, out_avals, zeros = [], [], [], []
    for alloc in nc.m.functions[0].allocations:
        if not isinstance(alloc, mybir.MemoryLocationSet):
            continue
        name = alloc.memorylocations[0].name
        if alloc.kind == "ExternalInput":
            if name != part_name:
                in_names.append(name)
        elif alloc.kind == "ExternalOutput":
            out_names.append(name)
            shape = tuple(alloc.tensor_shape)
            dtype = mybir.dt.np(alloc.dtype)
            out_avals.append(jax.core.ShapedArray(shape, dtype))
            zeros.append(np.zeros(shape, dtype))
    n_params = len(in_names)
    all_names = in_names + out_names
    if part_name is not None:
        all_names = all_names + [part_name]

    def _body(*args):
        operands = list(args)
        if part_name is not None:
            operands.append(bass2jax.partition_id_tensor())
        outs = bass2jax._bass_exec_p.bind(
            *operands,
            out_avals=tuple(out_avals),
            in_names=tuple(all_names),
            out_names=tuple(out_names),
            lowering_input_output_aliases=(),
            sim_require_finite=True,
            sim_require_nnan=True,
            nc=nc,
        )
        return tuple(outs)

    devices = jax.devices()[:NCORES]
    mesh = Mesh(np.asarray(devices), ("core",))
    nin = n_params + len(out_names)
    fn = jax.jit(shard_map(
        _body, mesh=mesh,
        in_specs=(PartitionSpec("core"),) * nin,
        out_specs=(PartitionSpec("core"),) * len(out_names),
        check_rep=False), keep_unused=True)

    sharding = jax.sharding.NamedSharding(mesh, PartitionSpec("core"))
    args = []
    for i, name in enumerate(in_names):
        cat = np.concatenate([np.asarray(m[name]) for m in in_maps], axis=0)
        args.append(jax.device_put(cat, sharding))
    for z in zeros:
        cat = np.zeros((NCORES * z.shape[0], *z.shape[1:]), z.dtype)
        args.append(jax.device_put(cat, sharding))

    out = fn(*args)          # warmup / compile
    jax.block_until_ready(out)
    out = fn(*args)
    jax.block_until_ready(out)

    t0 = time.perf_counter()
    outs = [fn(*args) for _ in range(iters)]
    jax.block_until_ready(outs)
    batched = (time.perf_counter() - t0) / iters

    best = float("inf")
    for _ in range(8):
        t0 = time.perf_counter()
        jax.block_until_ready(fn(*args))
        best = min(best, time.perf_counter() - t0)

    return int(batched * 1e9), int(best * 1e9)


if __name__ == "__main__":
    rng = np.random.default_rng(0)
    x = rng.standard_normal((N_NODES, IN_DIM), dtype=f32)
    W = (rng.standard_normal((OUT_DIM, IN_DIM), dtype=f32) / np.sqrt(IN_DIM)).astype(f32)
    b = (rng.standard_normal(OUT_DIM, dtype=f32) * 0.01).astype(f32)
    w_edge = rng.random(N_EDGES, dtype=f32)
    src = rng.integers(0, N_NODES, N_EDGES, dtype=np.int64)
    dst = rng.integers(0, N_NODES, N_EDGES, dtype=np.int64)
    out = kernel(x=x, W=W, b=b, w_edge=w_edge, src=src, dst=dst)
    h = x @ W.T + b
    import scipy.sparse as sp  # noqa — may not exist; fallback below
    try:
        A = sp.coo_matrix((w_edge, (dst, src)), shape=(N_NODES, N_NODES)).tocsr()
        want = A @ h
    except Exception:
        want = np.zeros_like(h)
        np.add.at(want, dst, h[src] * w_edge[:, None])
    err = np.abs(out - want).max() / (np.abs(want).max() + 1e-9)
    print("rel err:", err)


# revision 30
# speedup vs baseline: 1.2271x; 1.0739x over previous
"""GCNConv (linear + edge-weighted gather + segment_sum) on 8 TRN2 NeuronCores.

Strategy (dst-sharded, per the 1D graph-partition hint):
- Phase 1: node rows sharded 8-way; each core computes h = x @ W.T + b for its
  6250 nodes (fp32 matmul, fp32 psum, bias add), casts to fp16 and AllGathers
  so every core holds the full h [50000, 256] in HBM.
- Phase 2: destination nodes are bin-packed into 50 bins/core (<=128 dsts per
  bin, edge counts balanced).  Edges land in 128-slot tiles (dst-segment runs,
  zero-weight padding).  dma_gather pulls h[src] rows (fp16, 512B each) into
  SBUF; a per-tile one-hot*weight matrix B1w [128,32] (host-built) reduces the
  128 gathered rows into <=32 segment partial sums on the TensorEngine; a
  second one-hot matrix B2 combines those chunk rows into the bin's 128 output
  rows.  int16 gather indices cap at 32767, so edges are split into a low
  stream (src < 32768) and a high stream (gather base offset 32768).
- Host work is layout only: sharding/transposes, edge sorting/binning, and
  scattering w_edge into the block-structured B1w/B2 operands.
"""

import sys

if "/opt/trn_rl_repo" not in sys.path:
    sys.path.insert(0, "/opt/trn_rl_repo")

import os

import numpy as np

N_NODES = 50000
N_EDGES = 800000
IN_DIM = 512
OUT_DIM = 256
NCORES = 8
NODES_PER_CORE = N_NODES // NCORES  # 6250
NB = 50                 # dst bins per core (each bin -> 128 output rows)
BINS = NCORES * NB      # 400
SPLIT = 32768           # int16 gather index limit
GCALL = int(os.environ.get("GCN_GCALL", "32"))  # tiles per dma_gather call
MAX_SEGS = 32           # segment columns per level-1 tile

f32 = np.float32
f16 = np.float16
i16 = np.int16


# ---------------------------------------------------------------- host prep

def _bin_pack(dst, is_h):
    """Assign each dst node to one of BINS bins (<=128 dsts each), balancing
    (low, high) edge counts.  Returns bin_of_node [N_NODES]."""
    l_cnt = np.bincount(dst[~is_h], minlength=N_NODES).astype(np.int64)
    h_cnt = np.bincount(dst[is_h], minlength=N_NODES).astype(np.int64)
    tot = l_cnt + h_cnt
    order = np.argsort(-tot, kind="stable")

    import heapq
    # heap of (load, nitems, bin_id); load balances total edges
    heap = [(0, 0, b) for b in range(BINS)]
    heapq.heapify(heap)
    bin_of = np.empty(N_NODES, dtype=np.int32)
    stash = []
    for node in order:
        while True:
            load, cnt, b = heapq.heappop(heap)
            if cnt < 128:
                break
            stash.append(None)  # full bin, drop it
        bin_of[node] = b
        heapq.heappush(heap, (load + int(tot[node]), cnt + 1, b))
    return bin_of, l_cnt, h_cnt


def _pack_tiles(seg_list):
    """Pack (m, idx_array, w_array) segments into 128-slot tiles.

    Returns list of tiles; each tile is (idx[128] int32, col[128] int8,
    w[128] f32, seg2m[32] int32 with -1 for unused).  Segments split freely at
    tile boundaries; a tile holds at most MAX_SEGS segments."""
    tiles = []
    cur_idx = np.zeros(128, np.int64)
    cur_col = np.zeros(128, np.int8)
    cur_w = np.zeros(128, f32)
    cur_s2m = np.full(MAX_SEGS, -1, np.int32)
    pos = 0
    nseg = 0

    def close():
        nonlocal pos, nseg, cur_idx, cur_col, cur_w, cur_s2m
        tiles.append((cur_idx, cur_col, cur_w, cur_s2m))
        cur_idx = np.zeros(128, np.int64)
        cur_col = np.zeros(128, np.int8)
        cur_w = np.zeros(128, f32)
        cur_s2m = np.full(MAX_SEGS, -1, np.int32)
        pos = 0
        nseg = 0

    for m, idxs, ws in seg_list:
        off = 0
        n = len(idxs)
        while off < n:
            if pos == 128 or nseg == MAX_SEGS:
                close()
            take = min(n - off, 128 - pos)
            cur_idx[pos:pos + take] = idxs[off:off + take]
            cur_col[pos:pos + take] = nseg
            cur_w[pos:pos + take] = ws[off:off + take]
            cur_s2m[nseg] = m
            nseg += 1
            pos += take
            off += take
    if pos > 0 or nseg > 0:
        close()
    return tiles


def _prep(x, W, b, w_edge, src, dst):
    """All host-side sharding/layout. Returns (params, in_maps, unshard)."""
    src = np.asarray(src).astype(np.int64)
    dst = np.asarray(dst).astype(np.int64)
    w_edge = np.asarray(w_edge).astype(f32)
    is_h = src >= SPLIT

    bin_of, _, _ = _bin_pack(dst, is_h)

    # order edges by (bin, dst, stream)
    ekey = np.lexsort((is_h, dst, bin_of[dst]))
    e_src = src[ekey]
    e_dst = dst[ekey]
    e_w = w_edge[ekey]
    e_h = is_h[ekey]
    e_bin = bin_of[e_dst]

    # per-bin segment lists -> tiles; assign every node (incl. degree-0) a
    # local row m within its bin up front
    tiles_L = [[] for _ in range(BINS)]   # per bin: list of tile tuples
    tiles_H = [[] for _ in range(BINS)]
    m_of_node = np.full(N_NODES, -1, np.int32)
    nodes_of_bin = [[] for _ in range(BINS)]
    for node in np.argsort(bin_of, kind="stable"):
        bn = bin_of[node]
        m_of_node[node] = len(nodes_of_bin[bn])
        nodes_of_bin[bn].append(int(node))

    # group edges by (bin, dst, stream) using run boundaries
    key = e_bin.astype(np.int64) * (N_NODES * 2) + e_dst * 2 + e_h
    bounds = np.flatnonzero(np.r_[True, key[1:] != key[:-1], True])

    seglists_L = [[] for _ in range(BINS)]
    seglists_H = [[] for _ in range(BINS)]
    for gi in range(len(bounds) - 1):
        s, e = bounds[gi], bounds[gi + 1]
        bn = int(e_bin[s])
        node = int(e_dst[s])
        m = int(m_of_node[node])
        if e_h[s]:
            seglists_H[bn].append((m, e_src[s:e] - SPLIT, e_w[s:e]))
        else:
            seglists_L[bn].append((m, e_src[s:e], e_w[s:e]))

    for bn in range(BINS):
        tiles_L[bn] = _pack_tiles(seglists_L[bn])
        tiles_H[bn] = _pack_tiles(seglists_H[bn])

    T1L = max(1, max(len(t) for t in tiles_L))
    T1H = max(1, max(len(t) for t in tiles_H))
    T1 = T1L + T1H
    T2 = -(-T1 // 4)              # level-2 rhs tiles per bin (ceil T1/4)
    n_chunks = 32 * T1

    pad_tile = (np.zeros(128, np.int64), np.zeros(128, np.int8),
                np.zeros(128, f32), np.full(MAX_SEGS, -1, np.int32))

    # build per-core arrays
    in_maps = []
    node_perm = np.empty(N_NODES, np.int64)   # final out row -> original dst
    valid = np.zeros(NCORES * NB * 128, bool)

    xT = np.ascontiguousarray(x.T)            # [512, 50000] f32
    WT = np.ascontiguousarray(W.T)            # [512, 256] f32
    b2d = np.ascontiguousarray(b[None, :])    # [1, 256] f32

    ncall_L = -(-(NB * T1L) // GCALL)
    ncall_H = -(-(NB * T1H) // GCALL)

    for c in range(NCORES):
        b1w = np.zeros((NB, 128, T1 * MAX_SEGS), f16)
        b2m = np.zeros((NB, 128, T2 * 128), f16)
        idx_L = np.zeros((NB * T1L, 128), np.int16)
        idx_H = np.zeros((NB * T1H, 128), np.int16)

        for lb in range(NB):
            bn = c * NB + lb
            tl = tiles_L[bn] + [pad_tile] * (T1L - len(tiles_L[bn]))
            th = tiles_H[bn] + [pad_tile] * (T1H - len(tiles_H[bn]))
            for t, (tidx, tcol, tw, ts2m) in enumerate(tl + th):
                if t < T1L:
                    idx_L[lb * T1L + t] = tidx.astype(np.int16)
                else:
                    idx_H[lb * T1H + (t - T1L)] = tidx.astype(np.int16)
                cols = t * MAX_SEGS + tcol.astype(np.int32)
                b1w[lb, np.arange(128), cols] = tw.astype(f16)
                # chunk ids t*32+s -> m
                for s2 in range(MAX_SEGS):
                    m = ts2m[s2]
                    if m >= 0:
                        ch = t * MAX_SEGS + s2
                        b2m[lb, ch % 128, (ch // 128) * 128 + m] = 1.0

            for m, node in enumerate(nodes_of_bin[bn]):
                row = c * NB * 128 + lb * 128 + m
                node_perm[node] = row
                valid[row] = True

        def wrap_calls(idx_tiles, ncall):
            flat = idx_tiles.reshape(-1)           # [ntiles*128]
            out = np.zeros((ncall, 128, GCALL * 8), np.int16)
            for k in range(ncall):
                chunk = flat[k * GCALL * 128:(k + 1) * GCALL * 128]
                buf = np.zeros(GCALL * 128, np.int16)
                buf[:len(chunk)] = chunk
                wrapped = buf.reshape(-1, 16).T    # [16, GCALL*8]
                out[k] = np.tile(wrapped, (8, 1))
            return out

        rows = slice(c * NODES_PER_CORE, (c + 1) * NODES_PER_CORE)
        in_maps.append({
            "xT": np.ascontiguousarray(xT[:, rows]),
            "WT": WT,
            "bias": b2d,
            "idx_l": wrap_calls(idx_L, ncall_L),
            "idx_h": wrap_calls(idx_H, ncall_H),
            "b1w": b1w,
            "b2m": b2m,
        })

    params = dict(T1L=T1L, T1H=T1H, T1=T1, T2=T2, n_chunks=n_chunks,
                  ncall_L=ncall_L, ncall_H=ncall_H,
                  ntile_L=NB * T1L, ntile_H=NB * T1H)

    def unshard(outs):
        full = np.concatenate([o.reshape(-1, OUT_DIM) for o in outs], axis=0)
        return np.ascontiguousarray(full[node_perm])

    return params, in_maps, unshard


# ---------------------------------------------------------------- device

def _build(p):
    import os
    import concourse.bass as bass
    import concourse.mybir as mybir
    import concourse.tile as tile
    from concourse import bacc

    STAGE = int(os.environ.get("GCN_STAGE", "4"))

    dt16 = mybir.dt.float16
    dt32 = mybir.dt.float32
    dti16 = mybir.dt.int16

    T1L, T1H, T1, T2 = p["T1L"], p["T1H"], p["T1"], p["T2"]
    ncall_L, ncall_H = p["ncall_L"], p["ncall_H"]
    ntile_L, ntile_H = p["ntile_L"], p["ntile_H"]

    nc = bacc.Bacc(None, target_bir_lowering=False, num_swdge_queues=4)
    trace_sim = os.environ.get("GCN_TRACESIM", "0") == "1"

    xT_in = nc.dram_tensor("xT", [IN_DIM, NODES_PER_CORE], dt32, kind="ExternalInput")
    WT_in = nc.dram_tensor("WT", [IN_DIM, OUT_DIM], dt32, kind="ExternalInput")
    b_in = nc.dram_tensor("bias", [1, OUT_DIM], dt32, kind="ExternalInput")
    idxl_in = nc.dram_tensor("idx_l", [ncall_L, 128, GCALL * 8], dti16, kind="ExternalInput")
    idxh_in = nc.dram_tensor("idx_h", [ncall_H, 128, GCALL * 8], dti16, kind="ExternalInput")
    b1w_in = nc.dram_tensor("b1w", [NB, 128, T1 * MAX_SEGS], dt16, kind="ExternalInput")
    b2m_in = nc.dram_tensor("b2m", [NB, 128, T2 * 128], dt16, kind="ExternalInput")
    out_dr = nc.dram_tensor("out", [NB * 128, OUT_DIM], dt32, kind="ExternalOutput")

    h_loc = nc.dram_tensor("h_loc", [NODES_PER_CORE, OUT_DIM], dt16)
    h_all = nc.dram_tensor("h_all", [N_NODES, OUT_DIM], dt16, addr_space="Shared")

    NT = 49  # node tiles per core: 48*128 + 106 = 6250

    with tile.TileContext(nc, trace_sim=trace_sim) as tc:
        # ---------------- phase 1: h = x @ W.T + b (fp32), cast f16
        with (
            tc.tile_pool(name="p1", bufs=1) as p1,
            tc.tile_pool(name="p1x", bufs=1) as p1x,
            tc.tile_pool(name="p1h", bufs=4) as p1h,
            tc.tile_pool(name="ps1", bufs=2, space="PSUM") as ps1,
        ):
            wt_sb = []
            for k in range(4):
                t = p1.tile([128, OUT_DIM], dt16, tag=f"wt{k}")
                nc.gpsimd.dma_start(t[:], WT_in[128 * k:128 * (k + 1), :])
                wt_sb.append(t)
            bias_sb = p1.tile([128, OUT_DIM], dt32, tag="bias")
            nc.sync.dma_start(
                bias_sb[:],
                bass.AP(tensor=b_in.ap().tensor, offset=0,
                        ap=[[0, 128]] + [list(b_in.ap().ap[-1])]),
            )
            xt_big = []
            HALF = 25 * 128  # node boundary aligned to node-tiles
            for k in range(4):
                xt = p1x.tile([128, NODES_PER_CORE], dt16, tag=f"xt{k}")
                xt_big.append(xt)
            for k in range(4):   # first halves first: PE starts after these
                nc.gpsimd.dma_start(xt_big[k][:, :HALF],
                                    xT_in[128 * k:128 * (k + 1), :HALF])
            for k in range(4):
                nc.gpsimd.dma_start(xt_big[k][:, HALF:],
                                    xT_in[128 * k:128 * (k + 1), HALF:])
            for ntt in range(NT):
                w = min(128, NODES_PER_CORE - ntt * 128)
                hp = ps1.tile([128, OUT_DIM], dt32, tag="hps")
                for k in range(4):
                    nc.tensor.matmul(
                        hp[:w, :],
                        xt_big[k][:, ntt * 128:ntt * 128 + w],
                        wt_sb[k][:],
                        start=(k == 0), stop=(k == 3))
                hv = p1h.tile([128, OUT_DIM], dt16, tag="hv")
                nc.vector.tensor_add(hv[:w, :], hp[:w, :], bias_sb[:w, :])
                nc.sync.dma_start(h_loc[ntt * 128:ntt * 128 + w, :], hv[:w, :])

        # ---------------- AllGather
        if STAGE >= 1:
            nc.gpsimd.collective_compute(
                "AllGather",
                mybir.AluOpType.bypass,
                replica_groups=[list(range(NCORES))],
                ins=[h_loc.ap().opt()],
                outs=[h_all.ap().opt()],
            )

        # ---------------- phase 2: gather + two-level segment matmuls
        with (
            tc.tile_pool(name="gl", bufs=int(os.environ.get("GCN_GBUFS", "3"))) as gl_pool,
            tc.tile_pool(name="gh", bufs=int(os.environ.get("GCN_GBUFS", "3"))) as gh_pool,
            tc.tile_pool(name="ixp", bufs=3) as ix_pool,
            tc.tile_pool(name="bp", bufs=3) as b_pool,
            tc.tile_pool(name="ck", bufs=2 * (T2 + 1)) as ck_pool,
            tc.tile_pool(name="op", bufs=3) as out_pool,
            tc.tile_pool(name="ps2", bufs=3, space="PSUM") as ps2,
            tc.tile_pool(name="ps3", bufs=2, space="PSUM") as ps3,
        ):
            gtiles = {"L": [], "H": []}
            qrr = [0]

            def issue_gather(stream, k):
                ncall, ntile, idx_dr, base = {
                    "L": (ncall_L, ntile_L, idxl_in, 0),
                    "H": (ncall_H, ntile_H, idxh_in, SPLIT),
                }[stream]
                nt = min(GCALL, ntile - k * GCALL)
                it = ix_pool.tile([128, GCALL * 8], dti16, tag="ix")
                nc.scalar.dma_start(it[:], idx_dr[k, :, :])
                pool = gl_pool if stream == "L" else gh_pool
                gt = pool.tile([128, GCALL, OUT_DIM], dt16, tag="g" + stream)
                nc.gpsimd.dma_gather(
                    gt[:, :nt, :],
                    h_all[base:, :],
                    it[:, :nt * 8],
                    num_idxs=nt * 128,
                    num_idxs_reg=nt * 128,
                    elem_size=OUT_DIM,
                    single_packet=os.environ.get("GCN_SP", "0") == "1",
                    queue_num=qrr[0] % 4,
                )
                qrr[0] += 1
                gtiles[stream].append(gt)

            def get_tile_ap(stream, g):
                # slot data for global tile g of a stream
                return gtiles[stream][g // GCALL][:, g % GCALL, :]

            NB_RUN = NB if STAGE >= 2 else 0
            for lb in range(NB_RUN):
                # pull gather calls needed for this bin
                while len(gtiles["L"]) * GCALL < min((lb + 1) * T1L, ntile_L) \
                        or len(gtiles["L"]) == 0:
                    issue_gather("L", len(gtiles["L"]))
                while len(gtiles["H"]) * GCALL < min((lb + 1) * T1H, ntile_H) \
                        or len(gtiles["H"]) == 0:
                    issue_gather("H", len(gtiles["H"]))

                if STAGE == 2:
                    # just land the gathered tiles in out rows (garbage data,
                    # exercises gathers + DMA out)
                    ot = out_pool.tile([128, OUT_DIM], dt32, tag="ot")
                    g0 = gtiles["L"][(lb * T1L) // GCALL][:, (lb * T1L) % GCALL, :]
                    nc.vector.tensor_copy(ot[:], g0)
                    nc.sync.dma_start(out_dr[lb * 128:(lb + 1) * 128, :], ot[:])
                    continue

                b1t = b_pool.tile([128, T1 * MAX_SEGS], dt16, tag="b1")
                nc.scalar.dma_start(b1t[:], b1w_in[lb, :, :])
                b2t = b_pool.tile([128, T2 * 128], dt16, tag="b2")
                nc.scalar.dma_start(b2t[:], b2m_in[lb, :, :])

                # level 1
                cktiles = []
                for grp in range(T2):
                    g_n = min(4, T1 - grp * 4)
                    cps = ps2.tile([128, OUT_DIM], dt32, tag="cps")
                    for r in range(g_n):
                        t = grp * 4 + r
                        if t < T1L:
                            rhs = get_tile_ap("L", lb * T1L + t)
                        else:
                            rhs = get_tile_ap("H", lb * T1H + (t - T1L))
                        nc.tensor.matmul(
                            cps[32 * r:32 * (r + 1), :],
                            b1t[:, t * MAX_SEGS:(t + 1) * MAX_SEGS],
                            rhs,
                            start=True, stop=True,
                            tile_position=(0, 32 * r),
                        )
                    ckt = ck_pool.tile([128, OUT_DIM], dt16, tag="ck")
                    nc.any.tensor_copy(ckt[:32 * g_n, :], cps[:32 * g_n, :])
                    cktiles.append((ckt, 32 * g_n))

                if STAGE == 3:
                    ot = out_pool.tile([128, OUT_DIM], dt32, tag="ot")
                    nc.vector.tensor_copy(ot[:], cktiles[0][0][:])
                    nc.sync.dma_start(out_dr[lb * 128:(lb + 1) * 128, :], ot[:])
                    continue

                # level 2
                ops = ps3.tile([128, OUT_DIM], dt32, tag="ops")
                for j, (ckt, kk) in enumerate(cktiles):
                    nc.tensor.matmul(
                        ops[:],
                        b2t[:kk, j * 128:(j + 1) * 128],
                        ckt[:kk, :],
                        start=(j == 0), stop=(j == len(cktiles) - 1),
                    )
                ot = out_pool.tile([128, OUT_DIM], dt32, tag="ot")
                nc.any.tensor_copy(ot[:], ops[:])
                nc.sync.dma_start(out_dr[lb * 128:(lb + 1) * 128, :], ot[:])

    nc.compile()
    return nc


# ---------------------------------------------------------------- entry

TRACE = False          # test harness can flip this for neuron-profile timing
LAST_RESULT = None
_LAST_BUILD = None


def kernel(x, W, b, w_edge, src, dst):
    global LAST_RESULT, _LAST_BUILD
    from concourse.bass_utils import run_bass_kernel_spmd

    x = np.asarray(x, dtype=f32)
    W = np.asarray(W, dtype=f32)
    b = np.asarray(b, dtype=f32)

    params, in_maps, unshard = _prep(x, W, b, w_edge, src, dst)
    nc = _build(params)
    _LAST_BUILD = (nc, in_maps)
    res = run_bass_kernel_spmd(nc, in_maps, core_ids=list(range(NCORES)),
                               trace=TRACE)
    LAST_RESULT = res
    outs = [res.results[c]["out"] for c in range(NCORES)]
    return unshard(outs)


def bench(iters=32):
    """Time device-resident executions of the compiled kernel (no host I/O).

    Returns (batched_ns, min_iter_ns): batched = enqueue `iters` executions
    then sync once (pipelined, amortizes RPC); min_iter = best single
    dispatch+exec+sync round trip."""
    import time
    import jax
    from jax.sharding import Mesh, PartitionSpec
    from jax.experimental.shard_map import shard_map
    from concourse import bass2jax, mybir

    nc, in_maps = _LAST_BUILD
    bass2jax.install_neuronx_cc_hook()

    part_name = nc.partition_id_tensor.name if nc.partition_id_tensor else None
    in_names, out_names, out_avals, zeros = [], [], [], []
    for alloc in nc.m.functions[0].allocations:
        if not isinstance(alloc, mybir.MemoryLocationSet):
            continue
        name = alloc.memorylocations[0].name
        if alloc.kind == "ExternalInput":
            if name != part_name:
                in_names.append(name)
        elif alloc.kind == "ExternalOutput":
            out_names.append(name)
            shape = tuple(alloc.tensor_shape)
            dtype = mybir.dt.np(alloc.dtype)
            out_avals.append(jax.core.ShapedArray(shape, dtype))
            zeros.append(np.zeros(shape, dtype))
    n_params = len(in_names)
    all_names = in_names + out_names
    if part_name is not None:
        all_names = all_names + [part_name]

    def _body(*args):
        operands = list(args)
        if part_name is not None:
            operands.append(bass2jax.partition_id_tensor())
        outs = bass2jax._bass_exec_p.bind(
            *operands,
            out_avals=tuple(out_avals),
            in_names=tuple(all_names),
            out_names=tuple(out_names),
            lowering_input_output_aliases=(),
            sim_require_finite=True,
            sim_require_nnan=True,
            nc=nc,
        )
        return tuple(outs)

    devices = jax.devices()[:NCORES]
    mesh = Mesh(np.asarray(devices), ("core",))
    nin = n_params + len(out_names)
    fn = jax.jit(shard_map(
        _body, mesh=mesh,
        in_specs=(PartitionSpec("core"),) * nin,
        out_specs=(PartitionSpec("core"),) * len(out_names),
        check_rep=False), keep_unused=True)

    sharding = jax.sharding.NamedSharding(mesh, PartitionSpec("core"))
    args = []
    for i, name in enumerate(in_names):
        cat = np.concatenate([np.asarray(m[name]) for m in in_maps], axis=0)
        args.append(jax.device_put(cat, sharding))
    for z in zeros:
        cat = np.zeros((NCORES * z.shape[0], *z.shape[1:]), z.dtype)
        args.append(jax.device_put(cat, sharding))

    out = fn(*args)          # warmup / compile
    jax.block_until_ready(out)
    out = fn(*args)
    jax.block_until_ready(out)

    t0 = time.perf_counter()
    outs = [fn(*args) for _ in range(iters)]
    jax.block_until_ready(outs)
    batched = (time.perf_counter() - t0) / iters

    best = float("inf")
    for _ in range(8):
        t0 = time.perf_counter()
        jax.block_until_ready(fn(*args))
        best = min(best, time.perf_counter() - t0)

    return int(batched * 1e9), int(best * 1e9)


if __name__ == "__main__":
    rng = np.random.default_rng(0)
    x = rng.standard_normal((N_NODES, IN_DIM), dtype=f32)
    W = (rng.standard_normal((OUT_DIM, IN_DIM), dtype=f32) / np.sqrt(IN_DIM)).astype(f32)
    b = (rng.standard_normal(OUT_DIM, dtype=f32) * 0.01).astype(f32)
    w_edge = rng.random(N_EDGES, dtype=f32)
    src = rng.integers(0, N_NODES, N_EDGES, dtype=np.int64)
    dst = rng.integers(0, N_NODES, N_EDGES, dtype=np.int64)
    out = kernel(x=x, W=W, b=b, w_edge=w_edge, src=src, dst=dst)
    h = x @ W.T + b
    import scipy.sparse as sp  # noqa — may not exist; fallback below
    try:
        A = sp.coo_matrix((w_edge, (dst, src)), shape=(N_NODES, N_NODES)).tocsr()
        want = A @ h
    except Exception:
        want = np.zeros_like(h)
        np.add.at(want, dst, h[src] * w_edge[:, None])
    err = np.abs(out - want).max() / (np.abs(want).max() + 1e-9)
    print("rel err:", err)
